# revision 23
# baseline (speedup 1.0000x reference)
"""SmartLinearAppearance Trainium2 kernel.

Reference semantics (per (b, n) tracklet, reverse-time scan t = T-1 .. 0):
    xor  = (nv != 0) ^ (v_t != 0)
    prod = nv * v_t
    a_t  = prod * alpha + xor * nv          # per-part coefficient on state
    c_t  = prod * (1 - alpha) + xor * v_t   # per-part coefficient on input
    if m_t: ne = a_t[p] * ne + c_t[p] * e_t ; nv = max(nv, v_t)
    tok = where(any_t m, ne @ W.T + b, 0)

The recurrence is linear in embs given coefficients derived only from
(vis, masks), so it is reformulated as a single weighted reduction:
    ne[n, d] = sum_t w[n, t, p(d)] * embs[n, t, d]
    w = m * c * cumprod_{t' < t}(m ? a : 1)
Masked timesteps are exact no-ops of the recurrence, so valid timesteps
are compacted on the host (ragged -> padded to the global max valid
length TP) and embs is pre-cast to bf16, shrinking the HBM stream to
TP/T * 1/2 of the naive f32 read.

The per-tracklet coefficient chain runs on-device from the compacted
(vis, masks), with tracklet rows host-permuted to [evens | odds].  The
block-diagonal per-pair weight matrix is built on-chip: per part, the
two parity blocks are copied into a zeroed [64, 2*TP] staging tile in
block-diagonal form (partition-aligned copies at bases 0/32), and one
PE transpose yields the [2*TP, 64] rhs block at PSUM base 0, drained by
a single full-range DVE copy -- no DRAM round trip and no DMA on the
critical path.  Bias + final masking are folded into the stage-2 matmul
accumulation using a host-provided (any-mask) row.

Sharding: data-parallel over B across the 8 cores; the Linear weights
are replicated (pre-transposed/pre-tiled on the host).
"""

import sys

sys.path.insert(0, "/opt/trn_rl_repo")

import functools

import ml_dtypes
import numpy as np

import concourse.bacc as bacc
import concourse.bass as bass
import concourse.tile as tile
from concourse import mybir
from concourse.bass_utils import run_bass_kernel_spmd

B, N, T, D, V, TOK = 8, 64, 64, 1792, 7, 512
P = 7          # parts; F = D // P = 256
F = D // P
ALPHA = float(np.float32(0.9))
ONE_MINUS_ALPHA = float(np.float32(1.0) - np.float32(0.9))
NPAIR = N // 2           # 32 tracklet pairs per core
# embs DMA supertile sizes in pairs: big tiles amortize per-descriptor
# overhead; small tiles at the end shrink the post-stream tail
SGS = [4, 4, 4, 4, 4, 4, 4, 2, 1, 1]
DC = D // 128            # 14 d-chunks of 128

f32 = mybir.dt.float32
bf16 = mybir.dt.bfloat16


def _ap(t, offset_elems, dims):
    """Raw AP on a DRAM tensor/tile: dims = [[step, count], ...] in elements."""
    base = t[:] if hasattr(t, "shape") else t
    return bass.AP(tensor=base.tensor, offset=base.offset + offset_elems, ap=dims)


def build_nc(TP, debug=False):
    TVp = TP * V
    SH = [k for k in (1, 2, 4, 8, 16, 32) if k < TP]
    PAD = (SH[-1] if SH else 1) * V
    nc = bacc.Bacc()

    # host layout: supertiles [2(member), TP, PP(pair-in-tile), D], concat
    embs_c = nc.dram_tensor("embs_c", [NPAIR, 2 * TP * D], bf16,
                            kind="ExternalInput")
    vis_c = nc.dram_tensor("vis_c", [N, TVp], f32, kind="ExternalInput")
    mask_c = nc.dram_tensor("mask_c", [N, TP], f32, kind="ExternalInput")
    wt_c = nc.dram_tensor("wt_c", [128, DC * TOK], bf16, kind="ExternalInput")
    nmb_c = nc.dram_tensor("nmb_c", [1, N], bf16, kind="ExternalInput")
    bt_c = nc.dram_tensor("bt_c", [1, TOK], bf16, kind="ExternalInput")
    idn_c = nc.dram_tensor("idn_c", [64, 64], f32, kind="ExternalInput")
    out_c = nc.dram_tensor("out_c", [N, TOK], f32, kind="ExternalOutput")
    if debug:
        dbg_wco = nc.dram_tensor("dbg_wco", [N, TVp], f32, kind="ExternalOutput")
        dbg_wbd = nc.dram_tensor("dbg_wbd", [2 * TP, V * N], f32,
                                 kind="ExternalOutput")
        dbg_neT = nc.dram_tensor("dbg_neT", [128, DC * N], f32,
                                 kind="ExternalOutput")

    with tile.TileContext(nc) as tc:
        with (
            tc.tile_pool(name="small", bufs=1) as small,
            tc.tile_pool(name="big", bufs=1) as bigp,
            tc.tile_pool(name="embs", bufs=len(SGS)) as ep,
            tc.tile_pool(name="ps", bufs=1, space="PSUM") as ps,
            tc.tile_pool(name="pst", bufs=2, space="PSUM") as pst,
        ):
            # ---- wt on the scalar ring up front (independent family);
            # embs on the gpsimd ring with small inputs first so their
            # descriptors beat the embs flood into the queues ----
            wt_sb = bigp.tile([128, DC, TOK], bf16)
            nc.scalar.dma_start(
                out=wt_sb, in_=_ap(wt_c, 0, [[DC * TOK, 128], [1, DC * TOK]]))
            vis = small.tile([N, TVp], f32)
            nc.gpsimd.dma_start(out=vis, in_=vis_c[:, :])
            msk = small.tile([N, TP], f32)
            nc.gpsimd.dma_start(out=msk, in_=mask_c[:, :])
            nmb_sb = small.tile([1, N], bf16)
            nc.sync.dma_start(out=nmb_sb, in_=nmb_c[:, :])
            bt_sb = small.tile([1, TOK], bf16)
            nc.sync.dma_start(out=bt_sb, in_=bt_c[:, :])
            idn = small.tile([64, 64], f32)
            nc.sync.dma_start(out=idn, in_=idn_c[:, :])

            # embs stream: supertiles of SGS[i] pairs each
            ets = []
            qbase = 0
            for pp in SGS:
                et = ep.tile([2 * TP, pp, D], bf16)
                nc.gpsimd.dma_start(
                    out=et[:, :, :],
                    in_=_ap(embs_c, qbase * 2 * TP * D,
                            [[TP * pp * D, 2], [pp * D, TP], [1, pp * D]]))
                ets.append((et, qbase, pp))
                qbase += pp

            # mask broadcast view [N, TP, V] (step-0 inner dim)
            mb = bass.AP(tensor=msk.tensor, offset=msk.offset,
                         ap=[msk.ap[0][:], [1, TP], [0, V]])
            vis3 = vis.rearrange("n (t v) -> n t v", v=V)

            # ---- coefficient computation on [N, TP*V] ----
            mv = small.tile([N, TP, V], f32)
            nc.vector.tensor_tensor(out=mv, in0=vis3, in1=mb,
                                    op=mybir.AluOpType.mult)
            mvf = mv.rearrange("n t v -> n (t v)")

            # exclusive masked suffix max over t (log-doubling, zero pad)
            sA = small.tile([N, TVp + PAD], f32)
            sB = small.tile([N, TVp + PAD], f32)
            nc.vector.memset(sA, 0.0)
            nc.vector.memset(sB, 0.0)
            nc.vector.tensor_copy(out=sA[:, 0:TVp - V], in_=mvf[:, V:TVp])
            src, dst = sA, sB
            for k in SH:
                nc.vector.tensor_tensor(
                    out=dst[:, 0:TVp], in0=src[:, 0:TVp],
                    in1=src[:, k * V:k * V + TVp], op=mybir.AluOpType.max)
                src, dst = dst, src
            nv = src[:, 0:TVp]  # exclusive suffix max, [N, TP*V]

            n0 = small.tile([N, TVp], f32)
            nc.vector.tensor_scalar(out=n0, in0=nv, scalar1=0.0, scalar2=None,
                                    op0=mybir.AluOpType.is_gt)
            v0 = small.tile([N, TVp], f32)
            nc.vector.tensor_scalar(out=v0, in0=vis, scalar1=0.0, scalar2=None,
                                    op0=mybir.AluOpType.is_gt)
            xr = small.tile([N, TVp], f32)
            nc.vector.tensor_tensor(out=xr, in0=n0, in1=v0,
                                    op=mybir.AluOpType.not_equal)
            prod = small.tile([N, TVp], f32)
            nc.vector.tensor_tensor(out=prod, in0=nv, in1=vis,
                                    op=mybir.AluOpType.mult)
            xnv = small.tile([N, TVp], f32)
            nc.vector.tensor_tensor(out=xnv, in0=xr, in1=nv,
                                    op=mybir.AluOpType.mult)
            av = small.tile([N, TVp], f32)
            nc.vector.scalar_tensor_tensor(
                out=av, in0=prod, scalar=ALPHA, in1=xnv,
                op0=mybir.AluOpType.mult, op1=mybir.AluOpType.add)
            xv = small.tile([N, TVp], f32)
            nc.vector.tensor_tensor(out=xv, in0=xr, in1=vis,
                                    op=mybir.AluOpType.mult)
            cc = small.tile([N, TVp], f32)
            nc.vector.scalar_tensor_tensor(
                out=cc, in0=prod, scalar=ONE_MINUS_ALPHA, in1=xv,
                op0=mybir.AluOpType.mult, op1=mybir.AluOpType.add)

            # g = m * (a - 1) + 1, staged into gbuf with a leading slot of ones
            gb = small.tile([N, TVp + V], f32)
            nc.vector.memset(gb[:, 0:V], 1.0)
            av3 = av.rearrange("n (t v) -> n t v", v=V)
            gb3 = _ap(gb, V, [gb.ap[0][:], [V, TP], [1, V]])
            nc.vector.scalar_tensor_tensor(
                out=gb3, in0=av3, scalar=1.0, in1=mb,
                op0=mybir.AluOpType.subtract, op1=mybir.AluOpType.mult)
            nc.vector.tensor_scalar(out=gb[:, V:V + TVp], in0=gb[:, V:V + TVp],
                                    scalar1=1.0, scalar2=None,
                                    op0=mybir.AluOpType.add)

            # exclusive cumulative product over t per part (scan on data0 =
            # [1, g_0, ..., g_{TP-2}])
            pb = small.tile([N, TVp], f32)
            for p in range(V):
                dview = _ap(gb, p, [gb.ap[0][:], [V, TP]])
                oview = _ap(pb, p, [pb.ap[0][:], [V, TP]])
                nc.vector.tensor_tensor_scan(
                    out=oview, data0=dview, data1=dview, initial=1.0,
                    op0=mybir.AluOpType.mult, op1=mybir.AluOpType.bypass)

            mc = small.tile([N, TP, V], f32)
            nc.vector.tensor_tensor(
                out=mc, in0=cc.rearrange("n (t v) -> n t v", v=V), in1=mb,
                op=mybir.AluOpType.mult)
            wco = small.tile([N, TVp], f32)
            nc.vector.tensor_tensor(out=wco, in0=mc.rearrange("n t v -> n (t v)"),
                                    in1=pb, op=mybir.AluOpType.mult)
            wco3 = wco.rearrange("n (t v) -> n t v", v=V)

            # ---- block-diagonal weights built on-chip ----
            # Rows are host-permuted to [evens | odds].  Per part, copy the
            # two parity blocks into a zeroed [64, 2*TP] staging tile in
            # block-diagonal position (aligned partition bases 0 / 32), then
            # one PE transpose yields the [2*TP, 64] rhs block at PSUM base
            # 0, drained by one full-range DVE copy.
            in2a = small.tile([64, 2 * TP], f32)
            in2b = small.tile([64, 2 * TP], f32)
            nc.vector.memset(in2a, 0.0)
            nc.vector.memset(in2b, 0.0)
            in2 = [in2a, in2b]
            wbd = small.tile([2 * TP, V, N], bf16)
            for p in range(V):
                stg = in2[p % 2]
                nc.vector.tensor_copy(out=stg[0:32, 0:TP],
                                      in_=wco3[0:32, :, p])
                nc.vector.tensor_copy(out=stg[32:64, TP:2 * TP],
                                      in_=wco3[32:64, :, p])
                wtp = pst.tile([2 * TP, 64], f32)
                nc.tensor.transpose(out=wtp, in_=stg[:, :], identity=idn)
                nc.vector.tensor_copy(out=wbd[:, p, :], in_=wtp)

            # ---- stage 1: neT[d, n] = sum_t w[n, t, p(d)] * embs[n, t, d] ----
            # ping-pong PSUM tiles so the next supertile's matmuls overlap
            # the previous one's PSUM->SBUF drain
            neT_pp0 = ps.tile([128, DC, 8], f32)
            neT_pp1 = ps.tile([128, DC, 8], f32)
            neT_pp = [neT_pp0, neT_pp1]
            neT_sb = bigp.tile([128, DC, N], bf16)
            for g, (et, qbase, npp) in enumerate(ets):
                pp = neT_pp[g % 2]
                for jj in range(npp):
                    ip = qbase + jj
                    lhs_all = et[:, jj, :]
                    for dc in range(DC):
                        # rhs cols: perm positions (ip, 32+ip) = old (2ip, 2ip+1)
                        nc.tensor.matmul(
                            out=pp[:, dc, 2 * jj:2 * jj + 2],
                            lhsT=lhs_all[:, dc * 128:(dc + 1) * 128],
                            rhs=_ap(wbd, (dc // 2) * N + ip,
                                    [wbd.ap[0][:], [32, 2]]),
                            start=True, stop=True)
                nc.vector.tensor_copy(
                    out=neT_sb[:, :, 2 * qbase:2 * qbase + 2 * npp],
                    in_=pp[:, :, 0:2 * npp])

            # ---- stage 2: tok[n, k] = nm[n]*b[k] + sum_d neT[d, n]*wt[d, k] ----
            tok_ps = ps.tile([N, TOK], f32)
            nc.tensor.matmul(out=tok_ps, lhsT=nmb_sb, rhs=bt_sb,
                             start=True, stop=False)
            for dc in range(DC):
                nc.tensor.matmul(
                    out=tok_ps,
                    lhsT=neT_sb[:, dc, :],
                    rhs=wt_sb[:, dc, :],
                    start=False, stop=(dc == DC - 1))

            tok_sb = small.tile([N, TOK], f32)
            nc.vector.tensor_copy(out=tok_sb, in_=tok_ps)
            nc.sync.dma_start(out=out_c[:, :], in_=tok_sb)

            if debug:
                nc.sync.dma_start(out=dbg_wco[:, :], in_=wco)
                wbd_f = small.tile([2 * TP, V * N], f32)
                nc.vector.tensor_copy(
                    out=wbd_f, in_=wbd.rearrange("t a b -> t (a b)"))
                nc.sync.dma_start(out=dbg_wbd[:, :], in_=wbd_f)
                neT_f = small.tile([128, DC * N], f32)
                nc.vector.tensor_copy(
                    out=neT_f, in_=neT_sb.rearrange("d a b -> d (a b)"))
                nc.sync.dma_start(out=dbg_neT[:, :], in_=neT_f)

    nc.compile()
    return nc


@functools.lru_cache(maxsize=2)
def _get_nc(TP):
    return build_nc(TP)


def _prep_in_maps(embs, vis, masks, W, b):
    masks = np.asarray(masks)
    L = masks.sum(axis=2)                      # [B, N]
    TP = max(1, int(L.max()))

    # stable argsort of ~mask puts valid timesteps first, in t order
    order = np.argsort(~masks, axis=2, kind="stable")[:, :, :TP]  # [B,N,TP]

    embs_bf = np.asarray(embs).astype(ml_dtypes.bfloat16)
    embs_cmp = np.take_along_axis(embs_bf, order[..., None], axis=2)
    vis_cmp = np.take_along_axis(np.asarray(vis, np.float32),
                                 order[..., None], axis=2)  # [B,N,TP,V]
    mask_cmp = (np.arange(TP)[None, None, :] < L[..., None]).astype(np.float32)

    # row permutation for the chain: evens first, odds second
    perm = np.concatenate([np.arange(0, N, 2), np.arange(1, N, 2)])

    wt2 = np.ascontiguousarray(
        W.T.astype(ml_dtypes.bfloat16).reshape(DC, 128, TOK)
        .transpose(1, 0, 2).reshape(128, DC * TOK))
    bt = np.ascontiguousarray(b.astype(ml_dtypes.bfloat16)[None, :])
    idn = np.eye(64, dtype=np.float32)

    in_maps = []
    for c in range(B):
        # per supertile: [PP(pair), 2(member), TP, D] -> [2, TP, PP, D]
        ec = embs_cmp[c].reshape(NPAIR, 2, TP, D)
        parts = []
        qbase = 0
        for pp in SGS:
            parts.append(ec[qbase:qbase + pp].transpose(1, 2, 0, 3).reshape(-1))
            qbase += pp
        eg = np.ascontiguousarray(np.concatenate(parts)).reshape(NPAIR,
                                                                 2 * TP * D)
        nmb = (L[c] > 0).astype(ml_dtypes.bfloat16)[None, :]
        in_maps.append({
            "embs_c": eg,
            "vis_c": np.ascontiguousarray(vis_cmp[c][perm].reshape(N, TP * V)),
            "mask_c": np.ascontiguousarray(mask_cmp[c][perm]),
            "wt_c": wt2,
            "nmb_c": np.ascontiguousarray(nmb),
            "bt_c": bt,
            "idn_c": idn,
        })
    return TP, in_maps


def run(embs, vis, masks, W, b, **run_kwargs):
    TP, in_maps = _prep_in_maps(embs, vis, masks, W, b)
    nc = _get_nc(TP)
    res = run_bass_kernel_spmd(nc, in_maps, core_ids=list(range(B)),
                               **run_kwargs)
    out = np.stack([res.results[c]["out_c"] for c in range(B)], axis=0)
    return out, res


def kernel(embs, vis, masks, W, b):
    out, _ = run(embs, vis, masks, W, b)
    return out


# revision 24
# speedup vs baseline: 1.0168x; 1.0168x over previous
"""SmartLinearAppearance Trainium2 kernel.

Reference semantics (per (b, n) tracklet, reverse-time scan t = T-1 .. 0):
    xor  = (nv != 0) ^ (v_t != 0)
    prod = nv * v_t
    a_t  = prod * alpha + xor * nv          # per-part coefficient on state
    c_t  = prod * (1 - alpha) + xor * v_t   # per-part coefficient on input
    if m_t: ne = a_t[p] * ne + c_t[p] * e_t ; nv = max(nv, v_t)
    tok = where(any_t m, ne @ W.T + b, 0)

The recurrence is linear in embs given coefficients derived only from
(vis, masks), so it is reformulated as a single weighted reduction:
    ne[n, d] = sum_t w[n, t, p(d)] * embs[n, t, d]
    w = m * c * cumprod_{t' < t}(m ? a : 1)
Masked timesteps are exact no-ops of the recurrence, so valid timesteps
are compacted on the host (ragged -> padded to the global max valid
length TP) and embs is pre-cast to bf16, shrinking the HBM stream to
TP/T * 1/2 of the naive f32 read.

The per-tracklet coefficient chain runs on-device from the compacted
(vis, masks), with tracklet rows host-permuted to [evens | odds].  The
block-diagonal per-pair weight matrix is built on-chip: per part, the
two parity blocks are copied into a zeroed [64, 2*TP] staging tile in
block-diagonal form (partition-aligned copies at bases 0/32), and one
PE transpose yields the [2*TP, 64] rhs block at PSUM base 0, drained by
a single full-range DVE copy -- no DRAM round trip and no DMA on the
critical path.  Bias + final masking are folded into the stage-2 matmul
accumulation using a host-provided (any-mask) row.

Sharding: data-parallel over B across the 8 cores; the Linear weights
are replicated (pre-transposed/pre-tiled on the host).
"""

import sys

sys.path.insert(0, "/opt/trn_rl_repo")

import functools

import ml_dtypes
import numpy as np

import concourse.bacc as bacc
import concourse.bass as bass
import concourse.tile as tile
from concourse import mybir
from concourse.bass_utils import run_bass_kernel_spmd

B, N, T, D, V, TOK = 8, 64, 64, 1792, 7, 512
P = 7          # parts; F = D // P = 256
F = D // P
ALPHA = float(np.float32(0.9))
ONE_MINUS_ALPHA = float(np.float32(1.0) - np.float32(0.9))
NPAIR = N // 2           # 32 tracklet pairs per core
# embs DMA supertile sizes in pairs: big tiles amortize per-descriptor
# overhead; small tiles at the end shrink the post-stream tail
SGS = [4, 4, 4, 4, 4, 4, 4, 2, 1, 1]
DC = D // 128            # 14 d-chunks of 128

f32 = mybir.dt.float32
bf16 = mybir.dt.bfloat16


def _ap(t, offset_elems, dims):
    """Raw AP on a DRAM tensor/tile: dims = [[step, count], ...] in elements."""
    base = t[:] if hasattr(t, "shape") else t
    return bass.AP(tensor=base.tensor, offset=base.offset + offset_elems, ap=dims)


def build_nc(TP, debug=False):
    TVp = TP * V
    SH = [k for k in (1, 2, 4, 8, 16, 32) if k < TP]
    PAD = (SH[-1] if SH else 1) * V
    nc = bacc.Bacc()

    # host layout: supertiles [2(member), TP, PP(pair-in-tile), D], concat
    embs_c = nc.dram_tensor("embs_c", [NPAIR, 2 * TP * D], bf16,
                            kind="ExternalInput")
    vis_c = nc.dram_tensor("vis_c", [N, TVp], f32, kind="ExternalInput")
    mask_c = nc.dram_tensor("mask_c", [N, TP], f32, kind="ExternalInput")
    wt_c = nc.dram_tensor("wt_c", [128, DC * TOK], bf16, kind="ExternalInput")
    nmb_c = nc.dram_tensor("nmb_c", [1, N], bf16, kind="ExternalInput")
    bt_c = nc.dram_tensor("bt_c", [1, TOK], bf16, kind="ExternalInput")
    idn_c = nc.dram_tensor("idn_c", [64, 64], f32, kind="ExternalInput")
    out_c = nc.dram_tensor("out_c", [N, TOK], f32, kind="ExternalOutput")
    if debug:
        dbg_wco = nc.dram_tensor("dbg_wco", [N, TVp], f32, kind="ExternalOutput")
        dbg_wbd = nc.dram_tensor("dbg_wbd", [2 * TP, V * N], f32,
                                 kind="ExternalOutput")
        dbg_neT = nc.dram_tensor("dbg_neT", [128, DC * N], f32,
                                 kind="ExternalOutput")

    with tile.TileContext(nc) as tc:
        with (
            tc.tile_pool(name="small", bufs=1) as small,
            tc.tile_pool(name="big", bufs=1) as bigp,
            tc.tile_pool(name="embs", bufs=len(SGS)) as ep,
            tc.tile_pool(name="ps", bufs=1, space="PSUM") as ps,
            tc.tile_pool(name="pst", bufs=2, space="PSUM") as pst,
        ):
            # ---- wt on the sync ring up front; embs split across the
            # gpsimd (SWDGE) and scalar (HWDGE) families, whose backends
            # run concurrently; small inputs first on gpsimd so their
            # descriptors beat the embs flood ----
            wt_sb = bigp.tile([128, DC, TOK], bf16)
            nc.sync.dma_start(
                out=wt_sb, in_=_ap(wt_c, 0, [[DC * TOK, 128], [1, DC * TOK]]))
            vis = small.tile([N, TVp], f32)
            nc.gpsimd.dma_start(out=vis, in_=vis_c[:, :])
            msk = small.tile([N, TP], f32)
            nc.gpsimd.dma_start(out=msk, in_=mask_c[:, :])
            nmb_sb = small.tile([1, N], bf16)
            nc.sync.dma_start(out=nmb_sb, in_=nmb_c[:, :])
            bt_sb = small.tile([1, TOK], bf16)
            nc.sync.dma_start(out=bt_sb, in_=bt_c[:, :])
            idn = small.tile([64, 64], f32)
            nc.sync.dma_start(out=idn, in_=idn_c[:, :])

            # embs stream: supertiles of SGS[i] pairs, alternating between
            # the two DMA families
            ets = []
            qbase = 0
            for gi, pp in enumerate(SGS):
                et = ep.tile([2 * TP, pp, D], bf16)
                eng = nc.gpsimd if gi % 2 == 0 else nc.scalar
                eng.dma_start(
                    out=et[:, :, :],
                    in_=_ap(embs_c, qbase * 2 * TP * D,
                            [[TP * pp * D, 2], [pp * D, TP], [1, pp * D]]))
                ets.append((et, qbase, pp))
                qbase += pp

            # mask broadcast view [N, TP, V] (step-0 inner dim)
            mb = bass.AP(tensor=msk.tensor, offset=msk.offset,
                         ap=[msk.ap[0][:], [1, TP], [0, V]])
            vis3 = vis.rearrange("n (t v) -> n t v", v=V)

            # ---- coefficient computation on [N, TP*V] ----
            mv = small.tile([N, TP, V], f32)
            nc.vector.tensor_tensor(out=mv, in0=vis3, in1=mb,
                                    op=mybir.AluOpType.mult)
            mvf = mv.rearrange("n t v -> n (t v)")

            # exclusive masked suffix max over t (log-doubling, zero pad)
            sA = small.tile([N, TVp + PAD], f32)
            sB = small.tile([N, TVp + PAD], f32)
            nc.vector.memset(sA, 0.0)
            nc.vector.memset(sB, 0.0)
            nc.vector.tensor_copy(out=sA[:, 0:TVp - V], in_=mvf[:, V:TVp])
            src, dst = sA, sB
            for k in SH:
                nc.vector.tensor_tensor(
                    out=dst[:, 0:TVp], in0=src[:, 0:TVp],
                    in1=src[:, k * V:k * V + TVp], op=mybir.AluOpType.max)
                src, dst = dst, src
            nv = src[:, 0:TVp]  # exclusive suffix max, [N, TP*V]

            n0 = small.tile([N, TVp], f32)
            nc.vector.tensor_scalar(out=n0, in0=nv, scalar1=0.0, scalar2=None,
                                    op0=mybir.AluOpType.is_gt)
            v0 = small.tile([N, TVp], f32)
            nc.vector.tensor_scalar(out=v0, in0=vis, scalar1=0.0, scalar2=None,
                                    op0=mybir.AluOpType.is_gt)
            xr = small.tile([N, TVp], f32)
            nc.vector.tensor_tensor(out=xr, in0=n0, in1=v0,
                                    op=mybir.AluOpType.not_equal)
            prod = small.tile([N, TVp], f32)
            nc.vector.tensor_tensor(out=prod, in0=nv, in1=vis,
                                    op=mybir.AluOpType.mult)
            xnv = small.tile([N, TVp], f32)
            nc.vector.tensor_tensor(out=xnv, in0=xr, in1=nv,
                                    op=mybir.AluOpType.mult)
            av = small.tile([N, TVp], f32)
            nc.vector.scalar_tensor_tensor(
                out=av, in0=prod, scalar=ALPHA, in1=xnv,
                op0=mybir.AluOpType.mult, op1=mybir.AluOpType.add)
            xv = small.tile([N, TVp], f32)
            nc.vector.tensor_tensor(out=xv, in0=xr, in1=vis,
                                    op=mybir.AluOpType.mult)
            cc = small.tile([N, TVp], f32)
            nc.vector.scalar_tensor_tensor(
                out=cc, in0=prod, scalar=ONE_MINUS_ALPHA, in1=xv,
                op0=mybir.AluOpType.mult, op1=mybir.AluOpType.add)

            # g = m * (a - 1) + 1, staged into gbuf with a leading slot of ones
            gb = small.tile([N, TVp + V], f32)
            nc.vector.memset(gb[:, 0:V], 1.0)
            av3 = av.rearrange("n (t v) -> n t v", v=V)
            gb3 = _ap(gb, V, [gb.ap[0][:], [V, TP], [1, V]])
            nc.vector.scalar_tensor_tensor(
                out=gb3, in0=av3, scalar=1.0, in1=mb,
                op0=mybir.AluOpType.subtract, op1=mybir.AluOpType.mult)
            nc.vector.tensor_scalar(out=gb[:, V:V + TVp], in0=gb[:, V:V + TVp],
                                    scalar1=1.0, scalar2=None,
                                    op0=mybir.AluOpType.add)

            # exclusive cumulative product over t per part (scan on data0 =
            # [1, g_0, ..., g_{TP-2}])
            pb = small.tile([N, TVp], f32)
            for p in range(V):
                dview = _ap(gb, p, [gb.ap[0][:], [V, TP]])
                oview = _ap(pb, p, [pb.ap[0][:], [V, TP]])
                nc.vector.tensor_tensor_scan(
                    out=oview, data0=dview, data1=dview, initial=1.0,
                    op0=mybir.AluOpType.mult, op1=mybir.AluOpType.bypass)

            mc = small.tile([N, TP, V], f32)
            nc.vector.tensor_tensor(
                out=mc, in0=cc.rearrange("n (t v) -> n t v", v=V), in1=mb,
                op=mybir.AluOpType.mult)
            wco = small.tile([N, TVp], f32)
            nc.vector.tensor_tensor(out=wco, in0=mc.rearrange("n t v -> n (t v)"),
                                    in1=pb, op=mybir.AluOpType.mult)
            wco3 = wco.rearrange("n (t v) -> n t v", v=V)

            # ---- block-diagonal weights built on-chip ----
            # Rows are host-permuted to [evens | odds].  Per part, copy the
            # two parity blocks into a zeroed [64, 2*TP] staging tile in
            # block-diagonal position (aligned partition bases 0 / 32), then
            # one PE transpose yields the [2*TP, 64] rhs block at PSUM base
            # 0, drained by one full-range DVE copy.
            in2a = small.tile([64, 2 * TP], f32)
            in2b = small.tile([64, 2 * TP], f32)
            nc.vector.memset(in2a, 0.0)
            nc.vector.memset(in2b, 0.0)
            in2 = [in2a, in2b]
            wbd = small.tile([2 * TP, V, N], bf16)
            for p in range(V):
                stg = in2[p % 2]
                nc.vector.tensor_copy(out=stg[0:32, 0:TP],
                                      in_=wco3[0:32, :, p])
                nc.vector.tensor_copy(out=stg[32:64, TP:2 * TP],
                                      in_=wco3[32:64, :, p])
                wtp = pst.tile([2 * TP, 64], f32)
                nc.tensor.transpose(out=wtp, in_=stg[:, :], identity=idn)
                nc.vector.tensor_copy(out=wbd[:, p, :], in_=wtp)

            # ---- stage 1: neT[d, n] = sum_t w[n, t, p(d)] * embs[n, t, d] ----
            # ping-pong PSUM tiles so the next supertile's matmuls overlap
            # the previous one's PSUM->SBUF drain
            neT_pp0 = ps.tile([128, DC, 8], f32)
            neT_pp1 = ps.tile([128, DC, 8], f32)
            neT_pp = [neT_pp0, neT_pp1]
            neT_sb = bigp.tile([128, DC, N], bf16)
            for g, (et, qbase, npp) in enumerate(ets):
                pp = neT_pp[g % 2]
                for jj in range(npp):
                    ip = qbase + jj
                    lhs_all = et[:, jj, :]
                    for dc in range(DC):
                        # rhs cols: perm positions (ip, 32+ip) = old (2ip, 2ip+1)
                        nc.tensor.matmul(
                            out=pp[:, dc, 2 * jj:2 * jj + 2],
                            lhsT=lhs_all[:, dc * 128:(dc + 1) * 128],
                            rhs=_ap(wbd, (dc // 2) * N + ip,
                                    [wbd.ap[0][:], [32, 2]]),
                            start=True, stop=True)
                nc.vector.tensor_copy(
                    out=neT_sb[:, :, 2 * qbase:2 * qbase + 2 * npp],
                    in_=pp[:, :, 0:2 * npp])

            # ---- stage 2: tok[n, k] = nm[n]*b[k] + sum_d neT[d, n]*wt[d, k] ----
            tok_ps = ps.tile([N, TOK], f32)
            nc.tensor.matmul(out=tok_ps, lhsT=nmb_sb, rhs=bt_sb,
                             start=True, stop=False)
            for dc in range(DC):
                nc.tensor.matmul(
                    out=tok_ps,
                    lhsT=neT_sb[:, dc, :],
                    rhs=wt_sb[:, dc, :],
                    start=False, stop=(dc == DC - 1))

            tok_sb = small.tile([N, TOK], f32)
            nc.vector.tensor_copy(out=tok_sb, in_=tok_ps)
            nc.sync.dma_start(out=out_c[:, :], in_=tok_sb)

            if debug:
                nc.sync.dma_start(out=dbg_wco[:, :], in_=wco)
                wbd_f = small.tile([2 * TP, V * N], f32)
                nc.vector.tensor_copy(
                    out=wbd_f, in_=wbd.rearrange("t a b -> t (a b)"))
                nc.sync.dma_start(out=dbg_wbd[:, :], in_=wbd_f)
                neT_f = small.tile([128, DC * N], f32)
                nc.vector.tensor_copy(
                    out=neT_f, in_=neT_sb.rearrange("d a b -> d (a b)"))
                nc.sync.dma_start(out=dbg_neT[:, :], in_=neT_f)

    nc.compile()
    return nc


@functools.lru_cache(maxsize=2)
def _get_nc(TP):
    return build_nc(TP)


def _prep_in_maps(embs, vis, masks, W, b):
    masks = np.asarray(masks)
    L = masks.sum(axis=2)                      # [B, N]
    TP = max(1, int(L.max()))

    # stable argsort of ~mask puts valid timesteps first, in t order
    order = np.argsort(~masks, axis=2, kind="stable")[:, :, :TP]  # [B,N,TP]

    embs_bf = np.asarray(embs).astype(ml_dtypes.bfloat16)
    embs_cmp = np.take_along_axis(embs_bf, order[..., None], axis=2)
    vis_cmp = np.take_along_axis(np.asarray(vis, np.float32),
                                 order[..., None], axis=2)  # [B,N,TP,V]
    mask_cmp = (np.arange(TP)[None, None, :] < L[..., None]).astype(np.float32)

    # row permutation for the chain: evens first, odds second
    perm = np.concatenate([np.arange(0, N, 2), np.arange(1, N, 2)])

    wt2 = np.ascontiguousarray(
        W.T.astype(ml_dtypes.bfloat16).reshape(DC, 128, TOK)
        .transpose(1, 0, 2).reshape(128, DC * TOK))
    bt = np.ascontiguousarray(b.astype(ml_dtypes.bfloat16)[None, :])
    idn = np.eye(64, dtype=np.float32)

    in_maps = []
    for c in range(B):
        # per supertile: [PP(pair), 2(member), TP, D] -> [2, TP, PP, D]
        ec = embs_cmp[c].reshape(NPAIR, 2, TP, D)
        parts = []
        qbase = 0
        for pp in SGS:
            parts.append(ec[qbase:qbase + pp].transpose(1, 2, 0, 3).reshape(-1))
            qbase += pp
        eg = np.ascontiguousarray(np.concatenate(parts)).reshape(NPAIR,
                                                                 2 * TP * D)
        nmb = (L[c] > 0).astype(ml_dtypes.bfloat16)[None, :]
        in_maps.append({
            "embs_c": eg,
            "vis_c": np.ascontiguousarray(vis_cmp[c][perm].reshape(N, TP * V)),
            "mask_c": np.ascontiguousarray(mask_cmp[c][perm]),
            "wt_c": wt2,
            "nmb_c": np.ascontiguousarray(nmb),
            "bt_c": bt,
            "idn_c": idn,
        })
    return TP, in_maps


def run(embs, vis, masks, W, b, **run_kwargs):
    TP, in_maps = _prep_in_maps(embs, vis, masks, W, b)
    nc = _get_nc(TP)
    res = run_bass_kernel_spmd(nc, in_maps, core_ids=list(range(B)),
                               **run_kwargs)
    out = np.stack([res.results[c]["out_c"] for c in range(B)], axis=0)
    return out, res


def kernel(embs, vis, masks, W, b):
    out, _ = run(embs, vis, masks, W, b)
    return out


# revision 25
# speedup vs baseline: 1.0463x; 1.0290x over previous
"""SmartLinearAppearance Trainium2 kernel.

Reference semantics (per (b, n) tracklet, reverse-time scan t = T-1 .. 0):
    xor  = (nv != 0) ^ (v_t != 0)
    prod = nv * v_t
    a_t  = prod * alpha + xor * nv          # per-part coefficient on state
    c_t  = prod * (1 - alpha) + xor * v_t   # per-part coefficient on input
    if m_t: ne = a_t[p] * ne + c_t[p] * e_t ; nv = max(nv, v_t)
    tok = where(any_t m, ne @ W.T + b, 0)

The recurrence is linear in embs given coefficients derived only from
(vis, masks), so it is reformulated as a single weighted reduction:
    ne[n, d] = sum_t w[n, t, p(d)] * embs[n, t, d]
    w = m * c * cumprod_{t' < t}(m ? a : 1)
Masked timesteps are exact no-ops of the recurrence, so valid timesteps
are compacted on the host (ragged -> padded to the global max valid
length TP) and embs is pre-cast to bf16, shrinking the HBM stream to
TP/T * 1/2 of the naive f32 read.

Stage 1 contracts K tracklets per matmul (K = 3 when 3*TP <= 128) so
the embs stream covers K*TP of the 128 SBUF partitions -- DMA write
bandwidth scales with partitions covered, so K=3 (126 partitions) runs
~1.5x faster than pairs (84).  The block-diagonal weight matrix for the
K-tuples is built on-chip: the coefficient chain computes w with
tracklet rows placed at partition 32*pos + q (pos = position in tuple,
q = tuple index), a zeroed [CH, K*TP] staging tile collects the K
diagonal blocks via partition-aligned DVE copies (bases 0/32/64), and
one PE transpose per part yields the [K*TP, CH] rhs block at PSUM base
0.  Bias + final masking are folded into the stage-2 matmul
accumulation using a host-provided (any-mask) row.

Sharding: data-parallel over B across the 8 cores; the Linear weights
are replicated (pre-transposed/pre-tiled on the host).
"""

import sys

sys.path.insert(0, "/opt/trn_rl_repo")

import functools

import ml_dtypes
import numpy as np

import concourse.bacc as bacc
import concourse.bass as bass
import concourse.tile as tile
from concourse import mybir
from concourse.bass_utils import run_bass_kernel_spmd

B, N, T, D, V, TOK = 8, 64, 64, 1792, 7, 512
P = 7          # parts; F = D // P = 256
F = D // P
ALPHA = float(np.float32(0.9))
ONE_MINUS_ALPHA = float(np.float32(1.0) - np.float32(0.9))
DC = D // 128            # 14 d-chunks of 128

f32 = mybir.dt.float32
bf16 = mybir.dt.bfloat16


def _plan(TP):
    """Tuple size K, tuple count NT, chain partition count CH, supertile
    sizes (in tuples)."""
    K = 3 if 3 * TP <= 128 else 2
    NT = -(-N // K)                 # ceil
    CH = 32 * (K - 1) + NT
    if K == 3:
        SGS = [11, 7, 4]
    else:
        SGS = [16, 10, 6]
    assert sum(SGS) >= NT
    # trim to exactly NT tuples
    sgs, left = [], NT
    for s in SGS:
        s = min(s, left)
        if s > 0:
            sgs.append(s)
        left -= s
    return K, NT, CH, sgs


def build_nc(TP, debug=False):
    TVp = TP * V
    SH = [k for k in (1, 2, 4, 8, 16, 32) if k < TP]
    PAD = (SH[-1] if SH else 1) * V
    K, NT, CH, SGS = _plan(TP)
    KTP = K * TP
    nc = bacc.Bacc()

    # host layout: supertiles [K(pos), TP, ntr, D], concatenated
    embs_c = nc.dram_tensor("embs_c", [NT, K * TP * D], bf16,
                            kind="ExternalInput")
    vis_c = nc.dram_tensor("vis_c", [CH, TVp], f32, kind="ExternalInput")
    mask_c = nc.dram_tensor("mask_c", [CH, TP], f32, kind="ExternalInput")
    wt_c = nc.dram_tensor("wt_c", [128, DC * TOK], bf16, kind="ExternalInput")
    nmb_c = nc.dram_tensor("nmb_c", [1, N], bf16, kind="ExternalInput")
    bt_c = nc.dram_tensor("bt_c", [1, TOK], bf16, kind="ExternalInput")
    idn_c = nc.dram_tensor("idn_c", [CH, CH], f32, kind="ExternalInput")
    out_c = nc.dram_tensor("out_c", [N, TOK], f32, kind="ExternalOutput")
    if debug:
        dbg_wco = nc.dram_tensor("dbg_wco", [CH, TVp], f32,
                                 kind="ExternalOutput")
        dbg_wbd = nc.dram_tensor("dbg_wbd", [KTP, V * CH], f32,
                                 kind="ExternalOutput")
        dbg_neT = nc.dram_tensor("dbg_neT", [128, DC * K * NT], f32,
                                 kind="ExternalOutput")

    with tile.TileContext(nc) as tc:
        with (
            tc.tile_pool(name="small", bufs=1) as small,
            tc.tile_pool(name="big", bufs=1) as bigp,
            tc.tile_pool(name="embs", bufs=len(SGS)) as ep,
            tc.tile_pool(name="ps", bufs=1, space="PSUM") as ps,
            tc.tile_pool(name="pst", bufs=2, space="PSUM") as pst,
        ):
            # ---- wt on the sync ring; embs on the gpsimd ring with the
            # small chain inputs first so their descriptors beat the
            # embs flood into the queues ----
            wt_sb = bigp.tile([128, DC, TOK], bf16)
            nc.sync.dma_start(
                out=wt_sb, in_=_ap(wt_c, 0, [[DC * TOK, 128], [1, DC * TOK]]))
            vis = small.tile([CH, TVp], f32)
            nc.gpsimd.dma_start(out=vis, in_=vis_c[:, :])
            msk = small.tile([CH, TP], f32)
            nc.gpsimd.dma_start(out=msk, in_=mask_c[:, :])
            nmb_sb = small.tile([1, N], bf16)
            nc.sync.dma_start(out=nmb_sb, in_=nmb_c[:, :])
            bt_sb = small.tile([1, TOK], bf16)
            nc.sync.dma_start(out=bt_sb, in_=bt_c[:, :])
            idn = small.tile([CH, CH], f32)
            nc.sync.dma_start(out=idn, in_=idn_c[:, :])

            # embs stream: supertiles of SGS[i] K-tuples each
            ets = []
            qbase = 0
            for ntr in SGS:
                et = ep.tile([KTP, ntr, D], bf16)
                nc.gpsimd.dma_start(
                    out=et[:, :, :],
                    in_=_ap(embs_c, qbase * K * TP * D,
                            [[TP * ntr * D, K], [ntr * D, TP], [1, ntr * D]]))
                ets.append((et, qbase, ntr))
                qbase += ntr

            # mask broadcast view [CH, TP, V] (step-0 inner dim)
            mb = bass.AP(tensor=msk.tensor, offset=msk.offset,
                         ap=[msk.ap[0][:], [1, TP], [0, V]])
            vis3 = vis.rearrange("n (t v) -> n t v", v=V)

            # ---- coefficient computation on [CH, TP*V] ----
            mv = small.tile([CH, TP, V], f32)
            nc.vector.tensor_tensor(out=mv, in0=vis3, in1=mb,
                                    op=mybir.AluOpType.mult)
            mvf = mv.rearrange("n t v -> n (t v)")

            # exclusive masked suffix max over t (log-doubling, zero pad)
            sA = small.tile([CH, TVp + PAD], f32)
            sB = small.tile([CH, TVp + PAD], f32)
            nc.vector.memset(sA, 0.0)
            nc.vector.memset(sB, 0.0)
            nc.vector.tensor_copy(out=sA[:, 0:TVp - V], in_=mvf[:, V:TVp])
            src, dst = sA, sB
            for k in SH:
                nc.vector.tensor_tensor(
                    out=dst[:, 0:TVp], in0=src[:, 0:TVp],
                    in1=src[:, k * V:k * V + TVp], op=mybir.AluOpType.max)
                src, dst = dst, src
            nv = src[:, 0:TVp]  # exclusive suffix max, [CH, TP*V]

            n0 = small.tile([CH, TVp], f32)
            nc.vector.tensor_scalar(out=n0, in0=nv, scalar1=0.0, scalar2=None,
                                    op0=mybir.AluOpType.is_gt)
            v0 = small.tile([CH, TVp], f32)
            nc.vector.tensor_scalar(out=v0, in0=vis, scalar1=0.0, scalar2=None,
                                    op0=mybir.AluOpType.is_gt)
            xr = small.tile([CH, TVp], f32)
            nc.vector.tensor_tensor(out=xr, in0=n0, in1=v0,
                                    op=mybir.AluOpType.not_equal)
            prod = small.tile([CH, TVp], f32)
            nc.vector.tensor_tensor(out=prod, in0=nv, in1=vis,
                                    op=mybir.AluOpType.mult)
            xnv = small.tile([CH, TVp], f32)
            nc.vector.tensor_tensor(out=xnv, in0=xr, in1=nv,
                                    op=mybir.AluOpType.mult)
            av = small.tile([CH, TVp], f32)
            nc.vector.scalar_tensor_tensor(
                out=av, in0=prod, scalar=ALPHA, in1=xnv,
                op0=mybir.AluOpType.mult, op1=mybir.AluOpType.add)
            xv = small.tile([CH, TVp], f32)
            nc.vector.tensor_tensor(out=xv, in0=xr, in1=vis,
                                    op=mybir.AluOpType.mult)
            cc = small.tile([CH, TVp], f32)
            nc.vector.scalar_tensor_tensor(
                out=cc, in0=prod, scalar=ONE_MINUS_ALPHA, in1=xv,
                op0=mybir.AluOpType.mult, op1=mybir.AluOpType.add)

            # g = m * (a - 1) + 1, staged into gbuf with a leading slot of ones
            gb = small.tile([CH, TVp + V], f32)
            nc.vector.memset(gb[:, 0:V], 1.0)
            av3 = av.rearrange("n (t v) -> n t v", v=V)
            gb3 = _ap(gb, V, [gb.ap[0][:], [V, TP], [1, V]])
            nc.vector.scalar_tensor_tensor(
                out=gb3, in0=av3, scalar=1.0, in1=mb,
                op0=mybir.AluOpType.subtract, op1=mybir.AluOpType.mult)
            nc.vector.tensor_scalar(out=gb[:, V:V + TVp], in0=gb[:, V:V + TVp],
                                    scalar1=1.0, scalar2=None,
                                    op0=mybir.AluOpType.add)

            # exclusive cumulative product over t per part (scan on data0 =
            # [1, g_0, ..., g_{TP-2}])
            pb = small.tile([CH, TVp], f32)
            for p in range(V):
                dview = _ap(gb, p, [gb.ap[0][:], [V, TP]])
                oview = _ap(pb, p, [pb.ap[0][:], [V, TP]])
                nc.vector.tensor_tensor_scan(
                    out=oview, data0=dview, data1=dview, initial=1.0,
                    op0=mybir.AluOpType.mult, op1=mybir.AluOpType.bypass)

            mc = small.tile([CH, TP, V], f32)
            nc.vector.tensor_tensor(
                out=mc, in0=cc.rearrange("n (t v) -> n t v", v=V), in1=mb,
                op=mybir.AluOpType.mult)
            wco = small.tile([CH, TVp], f32)
            nc.vector.tensor_tensor(out=wco, in0=mc.rearrange("n t v -> n (t v)"),
                                    in1=pb, op=mybir.AluOpType.mult)
            wco3 = wco.rearrange("n (t v) -> n t v", v=V)

            # ---- block-diagonal weights built on-chip ----
            # Chain rows sit at 32*pos + q.  Per part, copy the K position
            # blocks into a zeroed [CH, K*TP] staging tile in block-diagonal
            # position (partition-aligned copies at bases 0/32/64), then one
            # PE transpose yields the [K*TP, CH] rhs block at PSUM base 0,
            # drained by one full-range DVE copy.
            in2a = small.tile([CH, KTP], f32)
            in2b = small.tile([CH, KTP], f32)
            nc.vector.memset(in2a, 0.0)
            nc.vector.memset(in2b, 0.0)
            in2 = [in2a, in2b]
            wbd = small.tile([KTP, V, CH], bf16)
            for p in range(V):
                stg = in2[p % 2]
                for pos in range(K):
                    nc.vector.tensor_copy(
                        out=stg[32 * pos:32 * pos + NT,
                                pos * TP:(pos + 1) * TP],
                        in_=wco3[32 * pos:32 * pos + NT, :, p])
                wtp = pst.tile([KTP, CH], f32)
                nc.tensor.transpose(out=wtp, in_=stg[:, :], identity=idn)
                nc.vector.tensor_copy(out=wbd[:, p, :], in_=wtp)

            # ---- stage 1: neT[d, n] = sum_t w[n, t, p(d)] * embs[n, t, d] ----
            # ping-pong PSUM tiles so the next supertile's matmuls overlap
            # the previous one's PSUM->SBUF drain
            max_ntr = max(SGS)
            neT_pp0 = ps.tile([128, DC, K * max_ntr], f32)
            neT_pp1 = ps.tile([128, DC, K * max_ntr], f32)
            neT_pp = [neT_pp0, neT_pp1]
            neT_sb = bigp.tile([128, DC, K * NT], bf16)
            for g, (et, qbase, ntr) in enumerate(ets):
                pp = neT_pp[g % 2]
                for j in range(ntr):
                    q = qbase + j
                    lhs_all = et[:, j, :]
                    for dc in range(DC):
                        # rhs cols {32*pos + q} = tuple q's K tracklets
                        nc.tensor.matmul(
                            out=pp[:, dc, K * j:K * j + K],
                            lhsT=lhs_all[:, dc * 128:(dc + 1) * 128],
                            rhs=_ap(wbd, (dc // 2) * CH + q,
                                    [wbd.ap[0][:], [32, K]]),
                            start=True, stop=True)
                nc.vector.tensor_copy(
                    out=neT_sb[:, :, K * qbase:K * (qbase + ntr)],
                    in_=pp[:, :, 0:K * ntr])

            # ---- stage 2: tok[n, k] = nm[n]*b[k] + sum_d neT[d, n]*wt[d, k] ----
            tok_ps = ps.tile([N, TOK], f32)
            nc.tensor.matmul(out=tok_ps, lhsT=nmb_sb, rhs=bt_sb,
                             start=True, stop=False)
            for dc in range(DC):
                nc.tensor.matmul(
                    out=tok_ps,
                    lhsT=neT_sb[:, dc, 0:N],
                    rhs=wt_sb[:, dc, :],
                    start=False, stop=(dc == DC - 1))

            tok_sb = small.tile([N, TOK], f32)
            nc.vector.tensor_copy(out=tok_sb, in_=tok_ps)
            nc.sync.dma_start(out=out_c[:, :], in_=tok_sb)

            if debug:
                nc.sync.dma_start(out=dbg_wco[:, :], in_=wco)
                wbd_f = small.tile([KTP, V * CH], f32)
                nc.vector.tensor_copy(
                    out=wbd_f, in_=wbd.rearrange("t a b -> t (a b)"))
                nc.sync.dma_start(out=dbg_wbd[:, :], in_=wbd_f)
                neT_f = small.tile([128, DC * K * NT], f32)
                nc.vector.tensor_copy(
                    out=neT_f, in_=neT_sb.rearrange("d a b -> d (a b)"))
                nc.sync.dma_start(out=dbg_neT[:, :], in_=neT_f)

    nc.compile()
    return nc


def _ap(t, offset_elems, dims):
    """Raw AP on a DRAM tensor/tile: dims = [[step, count], ...] in elements."""
    base = t[:] if hasattr(t, "shape") else t
    return bass.AP(tensor=base.tensor, offset=base.offset + offset_elems, ap=dims)


@functools.lru_cache(maxsize=2)
def _get_nc(TP):
    return build_nc(TP)


def _prep_in_maps(embs, vis, masks, W, b):
    masks = np.asarray(masks)
    L = masks.sum(axis=2)                      # [B, N]
    TP = max(1, int(L.max()))
    K, NT, CH, SGS = _plan(TP)

    # stable argsort of ~mask puts valid timesteps first, in t order
    order = np.argsort(~masks, axis=2, kind="stable")[:, :, :TP]  # [B,N,TP]

    embs_bf = np.asarray(embs).astype(ml_dtypes.bfloat16)
    embs_cmp = np.take_along_axis(embs_bf, order[..., None], axis=2)
    vis_cmp = np.take_along_axis(np.asarray(vis, np.float32),
                                 order[..., None], axis=2)  # [B,N,TP,V]
    mask_cmp = (np.arange(TP)[None, None, :] < L[..., None]).astype(np.float32)

    # tracklet of tuple q position pos: n = K*q + pos (clamped dummy -> 63)
    nidx = np.minimum(K * np.arange(NT)[:, None] + np.arange(K)[None, :],
                      N - 1)                   # [NT, K]
    dummy = (K * np.arange(NT)[:, None] + np.arange(K)[None, :]) > (N - 1)

    wt2 = np.ascontiguousarray(
        W.T.astype(ml_dtypes.bfloat16).reshape(DC, 128, TOK)
        .transpose(1, 0, 2).reshape(128, DC * TOK))
    bt = np.ascontiguousarray(b.astype(ml_dtypes.bfloat16)[None, :])
    idn = np.eye(CH, dtype=np.float32)

    in_maps = []
    for c in range(B):
        # chain rows at 32*pos + q; dummy rows zeroed
        vis96 = np.zeros((CH, TP * V), np.float32)
        msk96 = np.zeros((CH, TP), np.float32)
        for pos in range(K):
            for q in range(NT):
                if not dummy[q, pos]:
                    vis96[32 * pos + q] = vis_cmp[c, nidx[q, pos]].reshape(-1)
                    msk96[32 * pos + q] = mask_cmp[c, nidx[q, pos]]
        # embs supertiles: [K(pos), TP, ntr, D]
        parts = []
        qbase = 0
        for ntr in SGS:
            blk = embs_cmp[c][nidx[qbase:qbase + ntr]]   # [ntr, K, TP, D]
            parts.append(np.ascontiguousarray(
                blk.transpose(1, 2, 0, 3)).reshape(-1))
            qbase += ntr
        eg = np.concatenate(parts).reshape(NT, K * TP * D)
        nmb = (L[c] > 0).astype(ml_dtypes.bfloat16)[None, :]
        in_maps.append({
            "embs_c": eg,
            "vis_c": vis96,
            "mask_c": msk96,
            "wt_c": wt2,
            "nmb_c": np.ascontiguousarray(nmb),
            "bt_c": bt,
            "idn_c": idn,
        })
    return TP, in_maps


def run(embs, vis, masks, W, b, **run_kwargs):
    TP, in_maps = _prep_in_maps(embs, vis, masks, W, b)
    nc = _get_nc(TP)
    res = run_bass_kernel_spmd(nc, in_maps, core_ids=list(range(B)),
                               **run_kwargs)
    out = np.stack([res.results[c]["out_c"] for c in range(B)], axis=0)
    return out, res


def kernel(embs, vis, masks, W, b):
    out, _ = run(embs, vis, masks, W, b)
    return out


# revision 28
# speedup vs baseline: 1.1334x; 1.0833x over previous
"""SmartLinearAppearance Trainium2 kernel.

Reference semantics (per (b, n) tracklet, reverse-time scan t = T-1 .. 0):
    xor  = (nv != 0) ^ (v_t != 0)
    prod = nv * v_t
    a_t  = prod * alpha + xor * nv          # per-part coefficient on state
    c_t  = prod * (1 - alpha) + xor * v_t   # per-part coefficient on input
    if m_t: ne = a_t[p] * ne + c_t[p] * e_t ; nv = max(nv, v_t)
    tok = where(any_t m, ne @ W.T + b, 0)

The recurrence is linear in embs given coefficients derived only from
(vis, masks), so it is reformulated as a single weighted reduction:
    ne[n, d] = sum_t w[n, t, p(d)] * embs[n, t, d]
    w = m * c * cumprod_{t' < t}(m ? a : 1)
Masked timesteps are exact no-ops of the recurrence, so valid timesteps
are compacted on the host (ragged -> padded to the global max valid
length TP) and embs is pre-cast to bf16, shrinking the HBM stream to
TP/T * 1/2 of the naive f32 read.

Stage 1 contracts K tracklets per matmul (K = 3 when 3*TP <= 128) so
the embs stream covers K*TP of the 128 SBUF partitions -- DMA write
bandwidth scales with partitions covered, so K=3 (126 partitions) runs
~1.5x faster than pairs (84).  The block-diagonal weight matrix for the
K-tuples is built on-chip: the coefficient chain computes w with
tracklet rows placed at partition 32*pos + q (pos = position in tuple,
q = tuple index), a zeroed [CH, K*TP] staging tile collects the K
diagonal blocks via partition-aligned DVE copies (bases 0/32/64), and
one PE transpose per part yields the [K*TP, CH] rhs block at PSUM base
0.  Bias + final masking are folded into the stage-2 matmul
accumulation using a host-provided (any-mask) row.

Sharding: data-parallel over B across the 8 cores; the Linear weights
are replicated (pre-transposed/pre-tiled on the host).
"""

import sys

sys.path.insert(0, "/opt/trn_rl_repo")

import functools

import ml_dtypes
import numpy as np

import concourse.bacc as bacc
import concourse.bass as bass
import concourse.tile as tile
from concourse import mybir
from concourse.bass_utils import run_bass_kernel_spmd

B, N, T, D, V, TOK = 8, 64, 64, 1792, 7, 512
P = 7          # parts; F = D // P = 256
F = D // P
ALPHA = float(np.float32(0.9))
ONE_MINUS_ALPHA = float(np.float32(1.0) - np.float32(0.9))
DC = D // 128            # 14 d-chunks of 128

f32 = mybir.dt.float32
bf16 = mybir.dt.bfloat16


def _plan(TP):
    """Tuple size K, tuple count NT, chain partition count CH, supertile
    sizes (in tuples)."""
    K = 3 if 3 * TP <= 128 else 2
    NT = -(-N // K)                 # ceil
    CH = 32 * (K - 1) + NT
    if K == 3:
        SGS = [11, 7, 4]
    else:
        SGS = [16, 10, 6]
    assert sum(SGS) >= NT
    # trim to exactly NT tuples
    sgs, left = [], NT
    for s in SGS:
        s = min(s, left)
        if s > 0:
            sgs.append(s)
        left -= s
    return K, NT, CH, sgs


def build_nc(TP, debug=False):
    TVp = TP * V
    SH = [k for k in (1, 2, 4, 8, 16, 32) if k < TP]
    PAD = (SH[-1] if SH else 1) * V
    K, NT, CH, SGS = _plan(TP)
    KTP = K * TP
    nc = bacc.Bacc()

    # host layout: supertiles [K(pos), TP, ntr, D], concatenated
    embs_c = nc.dram_tensor("embs_c", [NT, K * TP * D], bf16,
                            kind="ExternalInput")
    vis_c = nc.dram_tensor("vis_c", [CH, TVp], f32, kind="ExternalInput")
    mask_c = nc.dram_tensor("mask_c", [CH, TP], f32, kind="ExternalInput")
    wt_c = nc.dram_tensor("wt_c", [128, DC * TOK], bf16, kind="ExternalInput")
    nmb_c = nc.dram_tensor("nmb_c", [1, N], bf16, kind="ExternalInput")
    bt_c = nc.dram_tensor("bt_c", [1, TOK], bf16, kind="ExternalInput")
    out_c = nc.dram_tensor("out_c", [N, TOK], f32, kind="ExternalOutput")
    if debug:
        dbg_wco = nc.dram_tensor("dbg_wco", [CH, TVp], f32,
                                 kind="ExternalOutput")
        dbg_wbd = nc.dram_tensor("dbg_wbd", [KTP, V * CH], f32,
                                 kind="ExternalOutput")
        dbg_neT = nc.dram_tensor("dbg_neT", [128, DC * K * NT], f32,
                                 kind="ExternalOutput")

    with tile.TileContext(nc) as tc:
        with (
            tc.tile_pool(name="small", bufs=1) as small,
            tc.tile_pool(name="big", bufs=1) as bigp,
            tc.tile_pool(name="embs", bufs=len(SGS)) as ep,
            tc.tile_pool(name="ps", bufs=1, space="PSUM") as ps,
            tc.tile_pool(name="pst", bufs=2, space="PSUM") as pst,
        ):
            # ---- single-family streaming on the gpsimd ring: small chain
            # inputs first (their descriptors beat the embs flood), then
            # the embs supertiles, then wt in two halves (so stage 2's
            # first chunks can start before the second half lands) ----
            vis = small.tile([CH, TVp], f32)
            nc.gpsimd.dma_start(out=vis, in_=vis_c[:, :])
            msk = small.tile([CH, TP], f32)
            nc.gpsimd.dma_start(out=msk, in_=mask_c[:, :])
            nmb_sb = small.tile([1, N], bf16)
            nc.sync.dma_start(out=nmb_sb, in_=nmb_c[:, :])
            bt_sb = small.tile([1, TOK], bf16)
            nc.sync.dma_start(out=bt_sb, in_=bt_c[:, :])

            # identity for the PE transposes, generated on-device
            idn_i = small.tile([CH, CH], mybir.dt.int32)
            nc.gpsimd.iota(idn_i, [[1, CH]], base=0, channel_multiplier=-1)
            idn = small.tile([CH, CH], f32)
            nc.gpsimd.tensor_scalar(out=idn, in0=idn_i, scalar1=0,
                                    scalar2=None,
                                    op0=mybir.AluOpType.is_equal)

            # embs stream: supertiles of SGS[i] K-tuples each
            ets = []
            qbase = 0
            for ntr in SGS:
                et = ep.tile([KTP, ntr, D], bf16)
                nc.gpsimd.dma_start(
                    out=et[:, :, :],
                    in_=_ap(embs_c, qbase * K * TP * D,
                            [[TP * ntr * D, K], [ntr * D, TP], [1, ntr * D]]))
                ets.append((et, qbase, ntr))
                qbase += ntr

            # wt after the embs supertiles, in two halves
            DCH = DC // 2
            wt_sb = bigp.tile([128, DC, TOK], bf16)
            nc.gpsimd.dma_start(
                out=wt_sb[:, 0:DCH, :],
                in_=_ap(wt_c, 0, [[DC * TOK, 128], [1, DCH * TOK]]))
            nc.gpsimd.dma_start(
                out=wt_sb[:, DCH:DC, :],
                in_=_ap(wt_c, DCH * TOK, [[DC * TOK, 128], [1, (DC - DCH) * TOK]]))

            # mask broadcast view [CH, TP, V] (step-0 inner dim)
            mb = bass.AP(tensor=msk.tensor, offset=msk.offset,
                         ap=[msk.ap[0][:], [1, TP], [0, V]])
            vis3 = vis.rearrange("n (t v) -> n t v", v=V)

            # ---- coefficient computation on [CH, TP*V] ----
            mv = small.tile([CH, TP, V], f32)
            nc.vector.tensor_tensor(out=mv, in0=vis3, in1=mb,
                                    op=mybir.AluOpType.mult)
            mvf = mv.rearrange("n t v -> n (t v)")

            # exclusive masked suffix max over t (log-doubling, zero pad)
            sA = small.tile([CH, TVp + PAD], f32)
            sB = small.tile([CH, TVp + PAD], f32)
            nc.vector.memset(sA, 0.0)
            nc.vector.memset(sB, 0.0)
            nc.vector.tensor_copy(out=sA[:, 0:TVp - V], in_=mvf[:, V:TVp])
            src, dst = sA, sB
            for k in SH:
                nc.vector.tensor_tensor(
                    out=dst[:, 0:TVp], in0=src[:, 0:TVp],
                    in1=src[:, k * V:k * V + TVp], op=mybir.AluOpType.max)
                src, dst = dst, src
            nv = src[:, 0:TVp]  # exclusive suffix max, [CH, TP*V]

            n0 = small.tile([CH, TVp], f32)
            nc.vector.tensor_scalar(out=n0, in0=nv, scalar1=0.0, scalar2=None,
                                    op0=mybir.AluOpType.is_gt)
            v0 = small.tile([CH, TVp], f32)
            nc.vector.tensor_scalar(out=v0, in0=vis, scalar1=0.0, scalar2=None,
                                    op0=mybir.AluOpType.is_gt)
            xr = small.tile([CH, TVp], f32)
            nc.vector.tensor_tensor(out=xr, in0=n0, in1=v0,
                                    op=mybir.AluOpType.not_equal)
            prod = small.tile([CH, TVp], f32)
            nc.vector.tensor_tensor(out=prod, in0=nv, in1=vis,
                                    op=mybir.AluOpType.mult)
            xnv = small.tile([CH, TVp], f32)
            nc.vector.tensor_tensor(out=xnv, in0=xr, in1=nv,
                                    op=mybir.AluOpType.mult)
            av = small.tile([CH, TVp], f32)
            nc.vector.scalar_tensor_tensor(
                out=av, in0=prod, scalar=ALPHA, in1=xnv,
                op0=mybir.AluOpType.mult, op1=mybir.AluOpType.add)
            xv = small.tile([CH, TVp], f32)
            nc.vector.tensor_tensor(out=xv, in0=xr, in1=vis,
                                    op=mybir.AluOpType.mult)
            cc = small.tile([CH, TVp], f32)
            nc.vector.scalar_tensor_tensor(
                out=cc, in0=prod, scalar=ONE_MINUS_ALPHA, in1=xv,
                op0=mybir.AluOpType.mult, op1=mybir.AluOpType.add)

            # g = m * (a - 1) + 1, staged into gbuf with a leading slot of ones
            gb = small.tile([CH, TVp + V], f32)
            nc.vector.memset(gb[:, 0:V], 1.0)
            av3 = av.rearrange("n (t v) -> n t v", v=V)
            gb3 = _ap(gb, V, [gb.ap[0][:], [V, TP], [1, V]])
            nc.vector.scalar_tensor_tensor(
                out=gb3, in0=av3, scalar=1.0, in1=mb,
                op0=mybir.AluOpType.subtract, op1=mybir.AluOpType.mult)
            nc.vector.tensor_scalar(out=gb[:, V:V + TVp], in0=gb[:, V:V + TVp],
                                    scalar1=1.0, scalar2=None,
                                    op0=mybir.AluOpType.add)

            # exclusive cumulative product over t per part (scan on data0 =
            # [1, g_0, ..., g_{TP-2}])
            pb = small.tile([CH, TVp], f32)
            for p in range(V):
                dview = _ap(gb, p, [gb.ap[0][:], [V, TP]])
                oview = _ap(pb, p, [pb.ap[0][:], [V, TP]])
                nc.vector.tensor_tensor_scan(
                    out=oview, data0=dview, data1=dview, initial=1.0,
                    op0=mybir.AluOpType.mult, op1=mybir.AluOpType.bypass)

            mc = small.tile([CH, TP, V], f32)
            nc.vector.tensor_tensor(
                out=mc, in0=cc.rearrange("n (t v) -> n t v", v=V), in1=mb,
                op=mybir.AluOpType.mult)
            wco = small.tile([CH, TVp], f32)
            nc.vector.tensor_tensor(out=wco, in0=mc.rearrange("n t v -> n (t v)"),
                                    in1=pb, op=mybir.AluOpType.mult)
            wco3 = wco.rearrange("n (t v) -> n t v", v=V)

            # ---- block-diagonal weights built on-chip ----
            # Chain rows sit at 32*pos + q.  Per part, copy the K position
            # blocks into a zeroed [CH, K*TP] staging tile in block-diagonal
            # position (partition-aligned copies at bases 0/32/64), then one
            # PE transpose yields the [K*TP, CH] rhs block at PSUM base 0,
            # drained by one full-range DVE copy.
            in2a = small.tile([CH, KTP], f32)
            in2b = small.tile([CH, KTP], f32)
            nc.vector.memset(in2a, 0.0)
            nc.vector.memset(in2b, 0.0)
            in2 = [in2a, in2b]
            wbd = small.tile([KTP, V, CH], bf16)
            for p in range(V):
                stg = in2[p % 2]
                for pos in range(K):
                    nc.vector.tensor_copy(
                        out=stg[32 * pos:32 * pos + NT,
                                pos * TP:(pos + 1) * TP],
                        in_=wco3[32 * pos:32 * pos + NT, :, p])
                wtp = pst.tile([KTP, CH], f32)
                nc.tensor.transpose(out=wtp, in_=stg[:, :], identity=idn)
                nc.vector.tensor_copy(out=wbd[:, p, :], in_=wtp)

            # ---- stage 1: neT[d, n] = sum_t w[n, t, p(d)] * embs[n, t, d] ----
            # ping-pong PSUM tiles so the next supertile's matmuls overlap
            # the previous one's PSUM->SBUF drain
            max_ntr = max(SGS)
            neT_pp0 = ps.tile([128, DC, K * max_ntr], f32)
            neT_pp1 = ps.tile([128, DC, K * max_ntr], f32)
            neT_pp = [neT_pp0, neT_pp1]
            neT_sb = bigp.tile([128, DC, K * NT], bf16)
            for g, (et, qbase, ntr) in enumerate(ets):
                pp = neT_pp[g % 2]
                for j in range(ntr):
                    q = qbase + j
                    lhs_all = et[:, j, :]
                    for dc in range(DC):
                        # rhs cols {32*pos + q} = tuple q's K tracklets
                        nc.tensor.matmul(
                            out=pp[:, dc, K * j:K * j + K],
                            lhsT=lhs_all[:, dc * 128:(dc + 1) * 128],
                            rhs=_ap(wbd, (dc // 2) * CH + q,
                                    [wbd.ap[0][:], [32, K]]),
                            start=True, stop=True)
                nc.vector.tensor_copy(
                    out=neT_sb[:, :, K * qbase:K * (qbase + ntr)],
                    in_=pp[:, :, 0:K * ntr])

            # ---- stage 2: tok[n, k] = nm[n]*b[k] + sum_d neT[d, n]*wt[d, k] ----
            tok_ps = ps.tile([N, TOK], f32)
            nc.tensor.matmul(out=tok_ps, lhsT=nmb_sb, rhs=bt_sb,
                             start=True, stop=False)
            for dc in range(DC):
                nc.tensor.matmul(
                    out=tok_ps,
                    lhsT=neT_sb[:, dc, 0:N],
                    rhs=wt_sb[:, dc, :],
                    start=False, stop=(dc == DC - 1))

            tok_sb = small.tile([N, TOK], f32)
            nc.vector.tensor_copy(out=tok_sb, in_=tok_ps)
            nc.sync.dma_start(out=out_c[:, :], in_=tok_sb)

            if debug:
                nc.sync.dma_start(out=dbg_wco[:, :], in_=wco)
                wbd_f = small.tile([KTP, V * CH], f32)
                nc.vector.tensor_copy(
                    out=wbd_f, in_=wbd.rearrange("t a b -> t (a b)"))
                nc.sync.dma_start(out=dbg_wbd[:, :], in_=wbd_f)
                neT_f = small.tile([128, DC * K * NT], f32)
                nc.vector.tensor_copy(
                    out=neT_f, in_=neT_sb.rearrange("d a b -> d (a b)"))
                nc.sync.dma_start(out=dbg_neT[:, :], in_=neT_f)

    nc.compile()
    return nc


def _ap(t, offset_elems, dims):
    """Raw AP on a DRAM tensor/tile: dims = [[step, count], ...] in elements."""
    base = t[:] if hasattr(t, "shape") else t
    return bass.AP(tensor=base.tensor, offset=base.offset + offset_elems, ap=dims)


@functools.lru_cache(maxsize=2)
def _get_nc(TP):
    return build_nc(TP)


def _prep_in_maps(embs, vis, masks, W, b):
    masks = np.asarray(masks)
    L = masks.sum(axis=2)                      # [B, N]
    TP = max(1, int(L.max()))
    K, NT, CH, SGS = _plan(TP)

    # stable argsort of ~mask puts valid timesteps first, in t order
    order = np.argsort(~masks, axis=2, kind="stable")[:, :, :TP]  # [B,N,TP]

    embs_bf = np.asarray(embs).astype(ml_dtypes.bfloat16)
    embs_cmp = np.take_along_axis(embs_bf, order[..., None], axis=2)
    vis_cmp = np.take_along_axis(np.asarray(vis, np.float32),
                                 order[..., None], axis=2)  # [B,N,TP,V]
    mask_cmp = (np.arange(TP)[None, None, :] < L[..., None]).astype(np.float32)

    # tracklet of tuple q position pos: n = K*q + pos (clamped dummy -> 63)
    nidx = np.minimum(K * np.arange(NT)[:, None] + np.arange(K)[None, :],
                      N - 1)                   # [NT, K]
    dummy = (K * np.arange(NT)[:, None] + np.arange(K)[None, :]) > (N - 1)

    wt2 = np.ascontiguousarray(
        W.T.astype(ml_dtypes.bfloat16).reshape(DC, 128, TOK)
        .transpose(1, 0, 2).reshape(128, DC * TOK))
    bt = np.ascontiguousarray(b.astype(ml_dtypes.bfloat16)[None, :])

    in_maps = []
    for c in range(B):
        # chain rows at 32*pos + q; dummy rows zeroed
        vis96 = np.zeros((CH, TP * V), np.float32)
        msk96 = np.zeros((CH, TP), np.float32)
        for pos in range(K):
            for q in range(NT):
                if not dummy[q, pos]:
                    vis96[32 * pos + q] = vis_cmp[c, nidx[q, pos]].reshape(-1)
                    msk96[32 * pos + q] = mask_cmp[c, nidx[q, pos]]
        # embs supertiles: [K(pos), TP, ntr, D]
        parts = []
        qbase = 0
        for ntr in SGS:
            blk = embs_cmp[c][nidx[qbase:qbase + ntr]]   # [ntr, K, TP, D]
            parts.append(np.ascontiguousarray(
                blk.transpose(1, 2, 0, 3)).reshape(-1))
            qbase += ntr
        eg = np.concatenate(parts).reshape(NT, K * TP * D)
        nmb = (L[c] > 0).astype(ml_dtypes.bfloat16)[None, :]
        in_maps.append({
            "embs_c": eg,
            "vis_c": vis96,
            "mask_c": msk96,
            "wt_c": wt2,
            "nmb_c": np.ascontiguousarray(nmb),
            "bt_c": bt,
        })
    return TP, in_maps


def run(embs, vis, masks, W, b, **run_kwargs):
    TP, in_maps = _prep_in_maps(embs, vis, masks, W, b)
    nc = _get_nc(TP)
    res = run_bass_kernel_spmd(nc, in_maps, core_ids=list(range(B)),
                               **run_kwargs)
    out = np.stack([res.results[c]["out_c"] for c in range(B)], axis=0)
    return out, res


def kernel(embs, vis, masks, W, b):
    out, _ = run(embs, vis, masks, W, b)
    return out
